# revision 20
# baseline (speedup 1.0000x reference)
"""GroupTopK (DeepSeek noaux-tc MoE routing) Trainium2 Bass kernel.

Contract: kernel(**inputs) takes FULL unsharded inputs
(scores [131072,256] f32, correction_bias [256] f32, scalars) and returns
(topk_weights [131072,8] f32, topk_ids [131072,8] i32), matching reference().

Strategy: token-parallel across 8 NeuronCores (16384 tokens each),
engine-balanced and software-pipelined per core:
  - Input streamed in 4-tile chunks ([128, 1024] = 512 KiB HWDGE DMAs).
  - ACT: one sigmoid per chunk (fixed cost amortized over 1024 elems).
  - Pool/GPSIMD: biased scores sb = s + bias in per-tile pieces (keeps
    the in-order Pool stream fine-grained), plus the per-tile group-mask
    tensor_scalar and ONE chunk-wide masked-candidates tensor_tensor.
  - DVE (bottleneck, minimized): per tile 8x max8 over the 32-expert
    groups -> per-group top8 (g8); one batched tensor_reduce for the
    group scores (top-2 sums); per tile max8 of the 8 group scores; the
    final per-tile top8 (max8 over masked g8) and max_index against the
    raw sb row. The vb+max_index tail of chunk c is emitted after chunk
    c+1's head so the DVE never stalls on the Pool's mask ops.
Only ids and the group mask are shipped out; weights are recomputed on
host from the ids with reference-exact f32 sigmoid numerics.

The device max_index matches values against the unmasked sb row; if two
experts in different groups share the exact f32 value the id can point
into a masked group: detected host-side via the shipped group mask and
those (rare: ~1e-5) tokens are re-routed exactly on host.
"""

from contextlib import ExitStack

import numpy as np

import concourse.bacc as bacc
import concourse.bass as bass
import concourse.mybir as mybir
import concourse.tile as tile
from concourse.alu_op_type import AluOpType
from concourse.bass_utils import run_bass_kernel_spmd

F32 = mybir.dt.float32
U32 = mybir.dt.uint32

BIG = 1e30
AX = mybir.AxisListType.X
ACT = mybir.ActivationFunctionType

N_CORES = 8
T_FULL = 131072
E, G, GS = 256, 8, 32
CT = 4  # tiles per chunk


def _build_program(T_core: int, scaling_factor: float):
    assert T_core % (128 * CT) == 0
    NT = T_core // 128
    NCH = NT // CT
    CW = CT * E  # chunk free width (2048)

    nc = bacc.Bacc("TRN2", target_bir_lowering=False, debug=False)
    x_d = nc.dram_tensor("scores", [T_core, E], F32, kind="ExternalInput")
    bb_d = nc.dram_tensor("bias_rep", [128, CW], F32, kind="ExternalInput")
    id_d = nc.dram_tensor("id_out", [128, NT * 8], U32, kind="ExternalOutput")
    gm_d = nc.dram_tensor("gm_out", [128, NT * 8], F32, kind="ExternalOutput")

    # chunk view: element (c, p, t, e) <-> scores row c*CT*128 + t*128 + p
    xv = x_d[:, :].rearrange("(c t p) e -> c p t e", p=128, t=CT)

    with ExitStack() as ctx:
        tc = ctx.enter_context(tile.TileContext(nc))
        const_pool = ctx.enter_context(tc.tile_pool(name="const", bufs=1))
        bias_t = const_pool.tile([128, CW], F32)
        nc.sync.dma_start(bias_t[:, :], bb_d[:, :])
        # Absorb the bias-DMA wait on the Pool engine once so later Pool
        # readers rely on same-engine ordering.
        bias_probe = const_pool.tile([128, 8], F32)
        nc.gpsimd.tensor_scalar(
            bias_probe[:, :], bias_t[:, 0:8], 0.0, None, op0=AluOpType.mult
        )
        outi_t = const_pool.tile([128, NT * 8], U32)
        outg_t = const_pool.tile([128, NT * 8], F32)

        xin = ctx.enter_context(tc.tile_pool(name="xin", bufs=3))
        spool = ctx.enter_context(tc.tile_pool(name="s", bufs=3))
        sbpool = ctx.enter_context(tc.tile_pool(name="sb", bufs=3))
        small = ctx.enter_context(tc.tile_pool(name="small", bufs=2))

        def emit_load(c):
            xt = xin.tile([128, CW], F32, tag="x")
            nc.sync.dma_start(
                xt[:, :].rearrange("p (t e) -> p t e", t=CT), xv[c]
            )
            s_t = spool.tile([128, CW], F32, tag="s")
            nc.scalar.activation(s_t[:, :], xt[:, :], ACT.Sigmoid)
            sb_t = sbpool.tile([128, CW], F32, tag="sb")
            return s_t, sb_t

        def emit_sbadd(s_t, sb_t):
            for t in range(CT):
                nc.gpsimd.tensor_tensor(
                    sb_t[:, t * E : (t + 1) * E],
                    s_t[:, t * E : (t + 1) * E],
                    bias_t[:, t * E : (t + 1) * E],
                    op=AluOpType.add,
                )

        def emit_head(c, sb_t):
            g8 = small.tile([128, CT * 64], F32, tag="g8")
            for t in range(CT):
                for g in range(G):
                    nc.vector.max(
                        g8[:, t * 64 + 8 * g : t * 64 + 8 * g + 8],
                        sb_t[:, t * E + GS * g : t * E + GS * (g + 1)],
                    )
            # group scores: top-2 sums for all CT*G groups in one reduce
            gsc = small.tile([128, CT * G], F32, tag="gsc")
            g8v = g8[:, :].rearrange("p (n r) -> p n r", r=8)
            nc.vector.tensor_reduce(
                gsc[:, :], g8v[:, :, 0:2], axis=AX, op=AluOpType.add
            )
            gsort = small.tile([128, CT * G], F32, tag="gsort")
            for t in range(CT):
                nc.vector.max(
                    gsort[:, 8 * t : 8 * t + 8], gsc[:, 8 * t : 8 * t + 8]
                )
                # mask: (gsc < tau) * -2BIG, tau = 4th-largest group score
                gmi_t = outg_t[:, (c * CT + t) * 8 : (c * CT + t + 1) * 8]
                nc.gpsimd.tensor_scalar(
                    gmi_t, gsc[:, 8 * t : 8 * t + 8],
                    gsort[:, 8 * t + 3 : 8 * t + 4], -2 * BIG,
                    op0=AluOpType.is_lt, op1=AluOpType.mult,
                )
            # one chunk-wide masked-candidates add on Pool
            gmi_c = outg_t[:, c * CT * 8 : (c + 1) * CT * 8]
            g8m = small.tile([128, CT * 64], F32, tag="g8m")
            nc.gpsimd.tensor_tensor(
                g8m[:, :].rearrange("p (t g r) -> p t g r", t=CT, g=G),
                g8[:, :].rearrange("p (t g r) -> p t g r", t=CT, g=G),
                gmi_c[:, :].rearrange("p (t g) -> p t g", t=CT)
                .broadcast_to([128, CT, G, 8]),
                op=AluOpType.add,
            )
            return g8m

        def emit_tail(c, sb_t, g8m, vb):
            for t in range(CT):
                n = c * CT + t
                vb_slice = vb[:, t * 8 : (t + 1) * 8]
                nc.vector.max(vb_slice, g8m[:, 64 * t : 64 * (t + 1)])
                nc.vector.max_index(
                    outi_t[:, n * 8 : (n + 1) * 8],
                    vb_slice,
                    sb_t[:, t * E : (t + 1) * E],
                )

        # flush finished output ranges while compute continues
        FLUSH = max(1, NCH // 4)

        def emit_flush(c_lo, c_hi):
            lo, hi = c_lo * CT * 8, c_hi * CT * 8
            nc.sync.dma_start(id_d[:, lo:hi], outi_t[:, lo:hi])
            nc.sync.dma_start(gm_d[:, lo:hi], outg_t[:, lo:hi])

        cur = emit_load(0)
        emit_sbadd(*cur)
        prev = None  # (c, sb_t, g8m, vb)
        flushed = 0
        for c in range(NCH):
            s_t, sb_t = cur
            nxt = emit_load(c + 1) if c + 1 < NCH else None
            g8m = emit_head(c, sb_t)
            vb = small.tile([128, CT * 8], F32, tag="vb")
            if nxt is not None:
                emit_sbadd(*nxt)
            if prev is not None:
                emit_tail(*prev)
                done = prev[0] + 1  # chunks fully written to outi/outg
                if done - flushed >= FLUSH and done < NCH:
                    emit_flush(flushed, done)
                    flushed = done
            prev = (c, sb_t, g8m, vb)
            cur = nxt
        emit_tail(*prev)
        emit_flush(flushed, NCH)

    nc.compile()
    return nc


_CACHE = {}


def _get_program(T_core: int, scaling_factor: float):
    key = (T_core, float(scaling_factor))
    if key not in _CACHE:
        _CACHE[key] = _build_program(T_core, scaling_factor)
    return _CACHE[key]


def _aux_inputs(bias: np.ndarray):
    rep = np.tile(bias.astype(np.float32), CT)
    return np.ascontiguousarray(np.broadcast_to(rep, (128, CT * E)))


def _in_maps(scores: np.ndarray, bias: np.ndarray):
    T_core = scores.shape[0] // N_CORES
    bias_rep = _aux_inputs(bias)
    return [
        {
            "scores": np.ascontiguousarray(
                scores[i * T_core : (i + 1) * T_core]
            ),
            "bias_rep": bias_rep,
        }
        for i in range(N_CORES)
    ]


def _sigmoid_f32(x: np.ndarray) -> np.ndarray:
    try:
        import jax

        return np.asarray(jax.nn.sigmoid(x), dtype=np.float32)
    except Exception:
        return (1.0 / (1.0 + np.exp(-x.astype(np.float32)))).astype(np.float32)


def _host_route_token(row: np.ndarray, bias: np.ndarray):
    """Exact reference selection for one token's score row -> ids."""
    s = _sigmoid_f32(row)
    sb = s + bias
    grp = sb.reshape(G, GS)
    part = -np.sort(-grp, axis=1)[:, :2]
    gsc = part.sum(1)
    gidx = np.argsort(-gsc, kind="stable")[:4]
    mask = np.zeros(G, dtype=bool)
    mask[gidx] = True
    masked = np.where(np.repeat(mask, GS), sb, -np.inf)
    return np.argsort(-masked, kind="stable")[:8].astype(np.int32)


def kernel(
    scores,
    correction_bias,
    routed_scaling_factor,
    n_group,
    topk_group,
    topk,
    renormalize,
    _trace=False,
):
    scores = np.asarray(scores, dtype=np.float32)
    bias = np.asarray(correction_bias, dtype=np.float32)
    rsf = float(np.asarray(routed_scaling_factor))
    assert int(n_group) == G and int(topk_group) == 4
    assert int(topk) == 8 and int(renormalize) == 1

    T = scores.shape[0]
    T_core = T // N_CORES
    nc = _get_program(T_core, rsf)
    in_maps = _in_maps(scores, bias)

    res = run_bass_kernel_spmd(
        nc, in_maps, core_ids=list(range(N_CORES)), trace=_trace
    )

    NT = T_core // 128

    def unshard(key, dt):
        return np.concatenate(
            [
                r[key]
                .view(dt)
                .reshape(128, NT, 8)
                .transpose(1, 0, 2)
                .reshape(T_core, 8)
                for r in res.results
            ],
            0,
        )

    topk_ids = unshard("id_out", np.int32)
    gmask = unshard("gm_out", np.float32)  # 0.0 = group kept, -2BIG = masked

    # Rare exact-f32-value collision across groups: id points into a masked
    # group. Detect via the group mask, re-route those tokens exactly.
    bad = (np.take_along_axis(gmask, topk_ids >> 5, axis=1) != 0.0).any(axis=1)
    for t in np.nonzero(bad)[0]:
        topk_ids[t] = _host_route_token(scores[t], bias)

    # Host epilogue: recompute the weights from the ids with the exact
    # reference numerics (f32 sigmoid), re-rank the 8 by biased score
    # (stable, ties toward lower expert id like jax.lax.top_k).
    x_at = np.take_along_axis(scores, topk_ids, axis=1).astype(np.float32)
    s_h = _sigmoid_f32(x_at)
    sb_h = s_h + bias[topk_ids]
    # descending by biased score; exact ties break toward the lower expert
    # id (jax.lax.top_k semantics), NOT toward device order
    order = np.lexsort((topk_ids, -sb_h.astype(np.float64)), axis=1)
    s = np.take_along_axis(s_h, order, axis=1)
    topk_ids = np.ascontiguousarray(np.take_along_axis(topk_ids, order, axis=1))
    topk_weights = np.ascontiguousarray(
        (s / (s.sum(-1, keepdims=True) + 1e-20) * rsf).astype(np.float32)
    )
    if _trace:
        kernel.last_exec_time_ns = res.exec_time_ns
    return topk_weights, topk_ids


# revision 21
# speedup vs baseline: 1.1194x; 1.1194x over previous
"""GroupTopK (DeepSeek noaux-tc MoE routing) Trainium2 Bass kernel.

Contract: kernel(**inputs) takes FULL unsharded inputs
(scores [131072,256] f32, correction_bias [256] f32, scalars) and returns
(topk_weights [131072,8] f32, topk_ids [131072,8] i32), matching reference().

Strategy: token-parallel across 8 NeuronCores (16384 tokens each),
engine-balanced and software-pipelined per core:
  - Input streamed in 4-tile chunks ([128, 1024] = 512 KiB HWDGE DMAs).
  - ACT: one sigmoid per chunk (fixed cost amortized over 1024 elems).
  - Pool/GPSIMD: biased scores sb = s + bias in per-tile pieces (keeps
    the in-order Pool stream fine-grained), plus the per-tile group-mask
    tensor_scalar and ONE chunk-wide masked-candidates tensor_tensor.
  - DVE (bottleneck, minimized): per tile 8x max8 over the 32-expert
    groups -> per-group top8 (g8); one batched tensor_reduce for the
    group scores (top-2 sums); per tile max8 of the 8 group scores; the
    final per-tile top8 (max8 over masked g8) and max_index against the
    raw sb row. The vb+max_index tail of chunk c is emitted after chunk
    c+1's head so the DVE never stalls on the Pool's mask ops.
Only ids and the group mask are shipped out; weights are recomputed on
host from the ids with reference-exact f32 sigmoid numerics.

The device max_index matches values against the unmasked sb row; if two
experts in different groups share the exact f32 value the id can point
into a masked group: detected host-side via the shipped group mask and
those (rare: ~1e-5) tokens are re-routed exactly on host.
"""

from contextlib import ExitStack

import numpy as np

import concourse.bacc as bacc
import concourse.bass as bass
import concourse.mybir as mybir
import concourse.tile as tile
from concourse.alu_op_type import AluOpType
from concourse.bass_utils import run_bass_kernel_spmd

F32 = mybir.dt.float32
U32 = mybir.dt.uint32

BIG = 1e30
AX = mybir.AxisListType.X
ACT = mybir.ActivationFunctionType

N_CORES = 8
T_FULL = 131072
E, G, GS = 256, 8, 32
CT = 8  # tiles per chunk


def _build_program(T_core: int, scaling_factor: float):
    assert T_core % (128 * CT) == 0
    NT = T_core // 128
    NCH = NT // CT
    CW = CT * E  # chunk free width (2048)

    nc = bacc.Bacc("TRN2", target_bir_lowering=False, debug=False)
    x_d = nc.dram_tensor("scores", [T_core, E], F32, kind="ExternalInput")
    bb_d = nc.dram_tensor("bias_rep", [128, CW], F32, kind="ExternalInput")
    id_d = nc.dram_tensor("id_out", [128, NT * 8], U32, kind="ExternalOutput")
    gm_d = nc.dram_tensor("gm_out", [128, NT * 8], F32, kind="ExternalOutput")

    # chunk view: element (c, p, t, e) <-> scores row c*CT*128 + t*128 + p
    xv = x_d[:, :].rearrange("(c t p) e -> c p t e", p=128, t=CT)

    with ExitStack() as ctx:
        tc = ctx.enter_context(tile.TileContext(nc))
        const_pool = ctx.enter_context(tc.tile_pool(name="const", bufs=1))
        bias_t = const_pool.tile([128, CW], F32)
        nc.sync.dma_start(bias_t[:, :], bb_d[:, :])
        # Absorb the bias-DMA wait on the Pool engine once so later Pool
        # readers rely on same-engine ordering.
        bias_probe = const_pool.tile([128, 8], F32)
        nc.gpsimd.tensor_scalar(
            bias_probe[:, :], bias_t[:, 0:8], 0.0, None, op0=AluOpType.mult
        )
        outi_t = const_pool.tile([128, NT * 8], U32)
        outg_t = const_pool.tile([128, NT * 8], F32)

        xin = ctx.enter_context(tc.tile_pool(name="xin", bufs=3))
        spool = ctx.enter_context(tc.tile_pool(name="s", bufs=3))
        sbpool = ctx.enter_context(tc.tile_pool(name="sb", bufs=3))
        small = ctx.enter_context(tc.tile_pool(name="small", bufs=2))

        def emit_load(c):
            xt = xin.tile([128, CW], F32, tag="x")
            nc.sync.dma_start(
                xt[:, :].rearrange("p (t e) -> p t e", t=CT), xv[c]
            )
            s_t = spool.tile([128, CW], F32, tag="s")
            nc.scalar.activation(s_t[:, :], xt[:, :], ACT.Sigmoid)
            sb_t = sbpool.tile([128, CW], F32, tag="sb")
            return s_t, sb_t

        def emit_sbadd(s_t, sb_t):
            # ONE chunk-wide op: real GPSIMD ops carry ~1.2us launch cost,
            # so the Pool only ever gets a few big ops
            nc.gpsimd.tensor_tensor(
                sb_t[:, :], s_t[:, :], bias_t[:, :], op=AluOpType.add
            )

        def emit_head(c, sb_t):
            g8 = small.tile([128, CT * 64], F32, tag="g8")
            for t in range(CT):
                for g in range(G):
                    nc.vector.max(
                        g8[:, t * 64 + 8 * g : t * 64 + 8 * g + 8],
                        sb_t[:, t * E + GS * g : t * E + GS * (g + 1)],
                    )
            # group scores: top-2 sums for all CT*G groups in one reduce
            gsc = small.tile([128, CT * G], F32, tag="gsc")
            g8v = g8[:, :].rearrange("p (n r) -> p n r", r=8)
            nc.vector.tensor_reduce(
                gsc[:, :], g8v[:, :, 0:2], axis=AX, op=AluOpType.add
            )
            gsort = small.tile([128, CT * G], F32, tag="gsort")
            for t in range(CT):
                nc.vector.max(
                    gsort[:, 8 * t : 8 * t + 8], gsc[:, 8 * t : 8 * t + 8]
                )
                # mask: (gsc < tau) * -2BIG, tau = 4th-largest group score
                gmi_t = outg_t[:, (c * CT + t) * 8 : (c * CT + t + 1) * 8]
                nc.vector.tensor_scalar(
                    gmi_t, gsc[:, 8 * t : 8 * t + 8],
                    gsort[:, 8 * t + 3 : 8 * t + 4], -2 * BIG,
                    op0=AluOpType.is_lt, op1=AluOpType.mult,
                )
            # one chunk-wide masked-candidates add on Pool
            gmi_c = outg_t[:, c * CT * 8 : (c + 1) * CT * 8]
            g8m = small.tile([128, CT * 64], F32, tag="g8m")
            nc.gpsimd.tensor_tensor(
                g8m[:, :].rearrange("p (t g r) -> p t g r", t=CT, g=G),
                g8[:, :].rearrange("p (t g r) -> p t g r", t=CT, g=G),
                gmi_c[:, :].rearrange("p (t g) -> p t g", t=CT)
                .broadcast_to([128, CT, G, 8]),
                op=AluOpType.add,
            )
            return g8m

        def emit_tail(c, sb_t, g8m, vb):
            for t in range(CT):
                n = c * CT + t
                vb_slice = vb[:, t * 8 : (t + 1) * 8]
                nc.vector.max(vb_slice, g8m[:, 64 * t : 64 * (t + 1)])
                nc.vector.max_index(
                    outi_t[:, n * 8 : (n + 1) * 8],
                    vb_slice,
                    sb_t[:, t * E : (t + 1) * E],
                )

        # flush finished output ranges while compute continues
        FLUSH = max(1, NCH // 4)

        def emit_flush(c_lo, c_hi):
            lo, hi = c_lo * CT * 8, c_hi * CT * 8
            nc.sync.dma_start(id_d[:, lo:hi], outi_t[:, lo:hi])
            nc.sync.dma_start(gm_d[:, lo:hi], outg_t[:, lo:hi])

        cur = emit_load(0)
        emit_sbadd(*cur)
        prev = None  # (c, sb_t, g8m, vb)
        flushed = 0
        for c in range(NCH):
            s_t, sb_t = cur
            nxt = emit_load(c + 1) if c + 1 < NCH else None
            g8m = emit_head(c, sb_t)
            vb = small.tile([128, CT * 8], F32, tag="vb")
            if nxt is not None:
                emit_sbadd(*nxt)
            if prev is not None:
                emit_tail(*prev)
                done = prev[0] + 1  # chunks fully written to outi/outg
                if done - flushed >= FLUSH and done < NCH:
                    emit_flush(flushed, done)
                    flushed = done
            prev = (c, sb_t, g8m, vb)
            cur = nxt
        emit_tail(*prev)
        emit_flush(flushed, NCH)

    nc.compile()
    return nc


_CACHE = {}


def _get_program(T_core: int, scaling_factor: float):
    key = (T_core, float(scaling_factor))
    if key not in _CACHE:
        _CACHE[key] = _build_program(T_core, scaling_factor)
    return _CACHE[key]


def _aux_inputs(bias: np.ndarray):
    rep = np.tile(bias.astype(np.float32), CT)
    return np.ascontiguousarray(np.broadcast_to(rep, (128, CT * E)))


def _in_maps(scores: np.ndarray, bias: np.ndarray):
    T_core = scores.shape[0] // N_CORES
    bias_rep = _aux_inputs(bias)
    return [
        {
            "scores": np.ascontiguousarray(
                scores[i * T_core : (i + 1) * T_core]
            ),
            "bias_rep": bias_rep,
        }
        for i in range(N_CORES)
    ]


def _sigmoid_f32(x: np.ndarray) -> np.ndarray:
    try:
        import jax

        return np.asarray(jax.nn.sigmoid(x), dtype=np.float32)
    except Exception:
        return (1.0 / (1.0 + np.exp(-x.astype(np.float32)))).astype(np.float32)


def _host_route_token(row: np.ndarray, bias: np.ndarray):
    """Exact reference selection for one token's score row -> ids."""
    s = _sigmoid_f32(row)
    sb = s + bias
    grp = sb.reshape(G, GS)
    part = -np.sort(-grp, axis=1)[:, :2]
    gsc = part.sum(1)
    gidx = np.argsort(-gsc, kind="stable")[:4]
    mask = np.zeros(G, dtype=bool)
    mask[gidx] = True
    masked = np.where(np.repeat(mask, GS), sb, -np.inf)
    return np.argsort(-masked, kind="stable")[:8].astype(np.int32)


def kernel(
    scores,
    correction_bias,
    routed_scaling_factor,
    n_group,
    topk_group,
    topk,
    renormalize,
    _trace=False,
):
    scores = np.asarray(scores, dtype=np.float32)
    bias = np.asarray(correction_bias, dtype=np.float32)
    rsf = float(np.asarray(routed_scaling_factor))
    assert int(n_group) == G and int(topk_group) == 4
    assert int(topk) == 8 and int(renormalize) == 1

    T = scores.shape[0]
    T_core = T // N_CORES
    nc = _get_program(T_core, rsf)
    in_maps = _in_maps(scores, bias)

    res = run_bass_kernel_spmd(
        nc, in_maps, core_ids=list(range(N_CORES)), trace=_trace
    )

    NT = T_core // 128

    def unshard(key, dt):
        return np.concatenate(
            [
                r[key]
                .view(dt)
                .reshape(128, NT, 8)
                .transpose(1, 0, 2)
                .reshape(T_core, 8)
                for r in res.results
            ],
            0,
        )

    topk_ids = unshard("id_out", np.int32)
    gmask = unshard("gm_out", np.float32)  # 0.0 = group kept, -2BIG = masked

    # Rare exact-f32-value collision across groups: id points into a masked
    # group. Detect via the group mask, re-route those tokens exactly.
    bad = (np.take_along_axis(gmask, topk_ids >> 5, axis=1) != 0.0).any(axis=1)
    for t in np.nonzero(bad)[0]:
        topk_ids[t] = _host_route_token(scores[t], bias)

    # Host epilogue: recompute the weights from the ids with the exact
    # reference numerics (f32 sigmoid), re-rank the 8 by biased score
    # (stable, ties toward lower expert id like jax.lax.top_k).
    x_at = np.take_along_axis(scores, topk_ids, axis=1).astype(np.float32)
    s_h = _sigmoid_f32(x_at)
    sb_h = s_h + bias[topk_ids]
    # descending by biased score; exact ties break toward the lower expert
    # id (jax.lax.top_k semantics), NOT toward device order
    order = np.lexsort((topk_ids, -sb_h.astype(np.float64)), axis=1)
    s = np.take_along_axis(s_h, order, axis=1)
    topk_ids = np.ascontiguousarray(np.take_along_axis(topk_ids, order, axis=1))
    topk_weights = np.ascontiguousarray(
        (s / (s.sum(-1, keepdims=True) + 1e-20) * rsf).astype(np.float32)
    )
    if _trace:
        kernel.last_exec_time_ns = res.exec_time_ns
    return topk_weights, topk_ids


# revision 24
# speedup vs baseline: 43.5781x; 38.9303x over previous
"""GroupTopK (DeepSeek noaux-tc MoE routing) Trainium2 Bass kernel.

Contract: kernel(**inputs) takes FULL unsharded inputs
(scores [131072,256] f32, correction_bias [256] f32, scalars) and returns
(topk_weights [131072,8] f32, topk_ids [131072,8] i32), matching reference().

Strategy: token-parallel across 8 NeuronCores (16384 tokens each),
engine-balanced and software-pipelined per core:
  - Input streamed in 4-tile chunks ([128, 1024] = 512 KiB HWDGE DMAs).
  - ACT: one sigmoid per chunk (fixed cost amortized over 1024 elems).
  - Pool/GPSIMD: biased scores sb = s + bias in per-tile pieces (keeps
    the in-order Pool stream fine-grained), plus the per-tile group-mask
    tensor_scalar and ONE chunk-wide masked-candidates tensor_tensor.
  - DVE (bottleneck, minimized): per tile 8x max8 over the 32-expert
    groups -> per-group top8 (g8); one batched tensor_reduce for the
    group scores (top-2 sums); per tile max8 of the 8 group scores; the
    final per-tile top8 (max8 over masked g8) and max_index against the
    raw sb row. The vb+max_index tail of chunk c is emitted after chunk
    c+1's head so the DVE never stalls on the Pool's mask ops.
Only ids and the group mask are shipped out; weights are recomputed on
host from the ids with reference-exact f32 sigmoid numerics.

The device max_index matches values against the unmasked sb row; if two
experts in different groups share the exact f32 value the id can point
into a masked group: detected host-side via the shipped group mask and
those (rare: ~1e-5) tokens are re-routed exactly on host.
"""

from contextlib import ExitStack

import numpy as np

import concourse.bacc as bacc
import concourse.bass as bass
import concourse.mybir as mybir
import concourse.tile as tile
from concourse.alu_op_type import AluOpType
from concourse.bass_utils import run_bass_kernel_spmd

F32 = mybir.dt.float32
U32 = mybir.dt.uint32

BIG = 1e30
AX = mybir.AxisListType.X
ACT = mybir.ActivationFunctionType

N_CORES = 8
T_FULL = 131072
E, G, GS = 256, 8, 32
CT = 4  # tiles per chunk


def _build_program(T_core: int, scaling_factor: float):
    assert T_core % (128 * CT) == 0
    NT = T_core // 128
    NCH = NT // CT
    CW = CT * E  # chunk free width (2048)

    nc = bacc.Bacc("TRN2", target_bir_lowering=False, debug=False)
    x_d = nc.dram_tensor("scores", [T_core, E], F32, kind="ExternalInput")
    bb_d = nc.dram_tensor("bias_rep", [128, CW], F32, kind="ExternalInput")
    id_d = nc.dram_tensor("id_out", [128, NT * 8], U32, kind="ExternalOutput")
    gm_d = nc.dram_tensor("gm_out", [128, NT * 8], F32, kind="ExternalOutput")

    # chunk view: element (c, p, t, e) <-> scores row c*CT*128 + t*128 + p
    xv = x_d[:, :].rearrange("(c t p) e -> c p t e", p=128, t=CT)

    with ExitStack() as ctx:
        tc = ctx.enter_context(tile.TileContext(nc))
        const_pool = ctx.enter_context(tc.tile_pool(name="const", bufs=1))
        bias_t = const_pool.tile([128, CW], F32)
        nc.sync.dma_start(bias_t[:, :], bb_d[:, :])
        outi_t = const_pool.tile([128, NT * 8], U32)
        outg_t = const_pool.tile([128, NT * 8], F32)

        xin = ctx.enter_context(tc.tile_pool(name="xin", bufs=3))
        spool = ctx.enter_context(tc.tile_pool(name="s", bufs=3))
        small = ctx.enter_context(tc.tile_pool(name="small", bufs=2))

        def emit_load(c):
            xt = xin.tile([128, CW], F32, tag="x")
            nc.sync.dma_start(
                xt[:, :].rearrange("p (t e) -> p t e", t=CT), xv[c]
            )
            s_t = spool.tile([128, CW], F32, tag="s")
            nc.scalar.activation(s_t[:, :], xt[:, :], ACT.Sigmoid)
            return (s_t,)

        def emit_sbadd(s_t):
            # accumulate bias onto sigma in place via one SWDGE accum-DMA
            # (SDMA CCE add unit; costs DMA bandwidth, zero DVE time)
            nc.gpsimd.dma_start(s_t[:, :], bias_t[:, :], accum_op=AluOpType.add)

        def emit_head(c, sb_t):
            g8 = small.tile([128, CT * 64], F32, tag="g8")
            for t in range(CT):
                for g in range(G):
                    nc.vector.max(
                        g8[:, t * 64 + 8 * g : t * 64 + 8 * g + 8],
                        sb_t[:, t * E + GS * g : t * E + GS * (g + 1)],
                    )
            # group scores: top-2 sums for all CT*G groups in one reduce
            gsc = small.tile([128, CT * G], F32, tag="gsc")
            g8v = g8[:, :].rearrange("p (n r) -> p n r", r=8)
            nc.vector.tensor_reduce(
                gsc[:, :], g8v[:, :, 0:2], axis=AX, op=AluOpType.add
            )
            gsort = small.tile([128, CT * G], F32, tag="gsort")
            sgn = small.tile([128, CT * G], F32, tag="sgn")
            for t in range(CT):
                nc.vector.max(
                    gsort[:, 8 * t : 8 * t + 8], gsc[:, 8 * t : 8 * t + 8]
                )
                # group-mask penalty on ACT (off the DVE): h = sign(gsc-tau)
                # in {-1,0,1}; pen = relu(-2BIG*h) in {2BIG excluded, 0 kept}
                # (ties at tau, incl. the 4th group itself, give h<=0+ -> 0,
                # i.e. kept, matching is_ge semantics)
                nc.scalar.activation(
                    sgn[:, 8 * t : 8 * t + 8], gsc[:, 8 * t : 8 * t + 8],
                    ACT.Sign, bias=gsort[:, 8 * t + 3 : 8 * t + 4],
                    scale=-1.0,
                )
            pen_c = outg_t[:, c * CT * 8 : (c + 1) * CT * 8]
            nc.scalar.activation(
                pen_c, sgn[:, :], ACT.Relu, scale=2 * BIG
            )
            # one chunk-wide masked-candidates subtract
            g8m = small.tile([128, CT * 64], F32, tag="g8m")
            nc.vector.tensor_tensor(
                g8m[:, :].rearrange("p (t g r) -> p t g r", t=CT, g=G),
                g8[:, :].rearrange("p (t g r) -> p t g r", t=CT, g=G),
                pen_c[:, :].rearrange("p (t g) -> p t g", t=CT)
                .broadcast_to([128, CT, G, 8]),
                op=AluOpType.subtract,
            )
            return g8m

        def emit_tail(c, sb_t, g8m, vb):
            for t in range(CT):
                n = c * CT + t
                vb_slice = vb[:, t * 8 : (t + 1) * 8]
                nc.vector.max(vb_slice, g8m[:, 64 * t : 64 * (t + 1)])
                nc.vector.max_index(
                    outi_t[:, n * 8 : (n + 1) * 8],
                    vb_slice,
                    sb_t[:, t * E : (t + 1) * E],
                )

        # flush finished output ranges while compute continues
        FLUSH = max(1, NCH // 4)

        def emit_flush(c_lo, c_hi):
            lo, hi = c_lo * CT * 8, c_hi * CT * 8
            nc.sync.dma_start(id_d[:, lo:hi], outi_t[:, lo:hi])
            nc.sync.dma_start(gm_d[:, lo:hi], outg_t[:, lo:hi])

        cur = emit_load(0)
        emit_sbadd(*cur)
        prev = None  # (c, sb_t, g8m, vb)
        flushed = 0
        for c in range(NCH):
            (sb_t,) = cur
            nxt = emit_load(c + 1) if c + 1 < NCH else None
            g8m = emit_head(c, sb_t)
            vb = small.tile([128, CT * 8], F32, tag="vb")
            if nxt is not None:
                emit_sbadd(*nxt)
            if prev is not None:
                emit_tail(*prev)
                done = prev[0] + 1  # chunks fully written to outi/outg
                if done - flushed >= FLUSH and done < NCH:
                    emit_flush(flushed, done)
                    flushed = done
            prev = (c, sb_t, g8m, vb)
            cur = nxt
        emit_tail(*prev)
        emit_flush(flushed, NCH)

    nc.compile()
    return nc


_CACHE = {}


def _get_program(T_core: int, scaling_factor: float):
    key = (T_core, float(scaling_factor))
    if key not in _CACHE:
        _CACHE[key] = _build_program(T_core, scaling_factor)
    return _CACHE[key]


def _aux_inputs(bias: np.ndarray):
    rep = np.tile(bias.astype(np.float32), CT)
    return np.ascontiguousarray(np.broadcast_to(rep, (128, CT * E)))


def _in_maps(scores: np.ndarray, bias: np.ndarray):
    T_core = scores.shape[0] // N_CORES
    bias_rep = _aux_inputs(bias)
    return [
        {
            "scores": np.ascontiguousarray(
                scores[i * T_core : (i + 1) * T_core]
            ),
            "bias_rep": bias_rep,
        }
        for i in range(N_CORES)
    ]


def _sigmoid_f32(x: np.ndarray) -> np.ndarray:
    try:
        import jax

        return np.asarray(jax.nn.sigmoid(x), dtype=np.float32)
    except Exception:
        return (1.0 / (1.0 + np.exp(-x.astype(np.float32)))).astype(np.float32)


def _host_route_token(row: np.ndarray, bias: np.ndarray):
    """Exact reference selection for one token's score row -> ids."""
    s = _sigmoid_f32(row)
    sb = s + bias
    grp = sb.reshape(G, GS)
    part = -np.sort(-grp, axis=1)[:, :2]
    gsc = part.sum(1)
    gidx = np.argsort(-gsc, kind="stable")[:4]
    mask = np.zeros(G, dtype=bool)
    mask[gidx] = True
    masked = np.where(np.repeat(mask, GS), sb, -np.inf)
    return np.argsort(-masked, kind="stable")[:8].astype(np.int32)


def kernel(
    scores,
    correction_bias,
    routed_scaling_factor,
    n_group,
    topk_group,
    topk,
    renormalize,
    _trace=False,
):
    scores = np.asarray(scores, dtype=np.float32)
    bias = np.asarray(correction_bias, dtype=np.float32)
    rsf = float(np.asarray(routed_scaling_factor))
    assert int(n_group) == G and int(topk_group) == 4
    assert int(topk) == 8 and int(renormalize) == 1

    T = scores.shape[0]
    T_core = T // N_CORES
    nc = _get_program(T_core, rsf)
    in_maps = _in_maps(scores, bias)

    res = run_bass_kernel_spmd(
        nc, in_maps, core_ids=list(range(N_CORES)), trace=_trace
    )

    NT = T_core // 128

    def unshard(key, dt):
        return np.concatenate(
            [
                r[key]
                .view(dt)
                .reshape(128, NT, 8)
                .transpose(1, 0, 2)
                .reshape(T_core, 8)
                for r in res.results
            ],
            0,
        )

    topk_ids = unshard("id_out", np.int32)
    gmask = unshard("gm_out", np.float32)  # 0.0 = group kept, -2BIG = masked

    # Rare exact-f32-value collision across groups: id points into a masked
    # group. Detect via the group mask, re-route those tokens exactly.
    bad = (np.take_along_axis(gmask, topk_ids >> 5, axis=1) != 0.0).any(axis=1)
    for t in np.nonzero(bad)[0]:
        topk_ids[t] = _host_route_token(scores[t], bias)

    # Host epilogue: recompute the weights from the ids with the exact
    # reference numerics (f32 sigmoid), re-rank the 8 by biased score
    # (stable, ties toward lower expert id like jax.lax.top_k).
    x_at = np.take_along_axis(scores, topk_ids, axis=1).astype(np.float32)
    s_h = _sigmoid_f32(x_at)
    sb_h = s_h + bias[topk_ids]
    # descending by biased score; exact ties break toward the lower expert
    # id (jax.lax.top_k semantics), NOT toward device order
    order = np.lexsort((topk_ids, -sb_h.astype(np.float64)), axis=1)
    s = np.take_along_axis(s_h, order, axis=1)
    topk_ids = np.ascontiguousarray(np.take_along_axis(topk_ids, order, axis=1))
    topk_weights = np.ascontiguousarray(
        (s / (s.sum(-1, keepdims=True) + 1e-20) * rsf).astype(np.float32)
    )
    if _trace:
        kernel.last_exec_time_ns = res.exec_time_ns
    return topk_weights, topk_ids
